# revision 59
# baseline (speedup 1.0000x reference)
"""Additive (Bahdanau) attention on 8 TRN2 NeuronCores.

Reference computation:
    qp = queries @ W_q                  (bs, n_q, 64)
    kp = keys @ W_k                     (bs, n_k, 64)
    scores[b,q,k] = sum_h w_v[h] * tanh(qp[b,q,h] + kp[b,k,h])
    out = softmax(scores, -1) @ values

Key trick: tanh(x) on [-9.9, 9.9] is approximated by a sum of J=6 sines
(odd harmonics of w0, max err 1.2e-2; end-to-end rel err 7.4e-3):
    tanh(x) ~= sum_j c_j sin(w_j x),  w_j = (2j+1) w0
Angle addition makes the score computation separable:
    sin(w(a+b)) = sin(wa)cos(wb) + cos(wa)sin(wb)
so scores reduce to matmuls with contraction 2*64 per harmonic — pure
TensorEngine work. The giant (bs, n_q, n_k, 64) tanh tensor of the naive
implementation never exists.

sin/cos args reach |w_j x| ~ 17 rad but the ScalarE Sin spline only covers
[-4, 4], so angles are range-reduced via fp32 bit surgery (j >= 1):
    z = x * (w_j / 2pi) + (S + 36)      # S = 0 (sin half) / 0.25 (cos half)
      -> z in [32, 64): exponent fixed at 5, so frac(z) is the low 18
         mantissa bits exactly.
    v = (bits(z) & 0x3FFFF) | bits(1.0) # v = 1 + frac(z) * 2^-23
    sin(2 pi z) = Sin(-64 pi * v + 65 pi)
Two 2x-rate tensor_scalars per harmonic per side, then one activation whose
own affine un-biases the fraction; no tensor_tensor ops, and the GpSimd
engine (which shares SBUF ports with the DVE) stays out of the data path.

Scores are built TRANSPOSED (k on partitions, q free) so the attention
weights feed the output matmul with no transposes:
    outT (v, q) = sum_kt values[kt] (lhsT) @ expT[kt]
    sums (1, q) = sum_kt ones^T @ expT[kt]
and only a final (v, q) -> (q, v) transpose + per-partition normalize
remain.

Sharding: fully data-parallel, no collectives. Core c handles batch c//2,
query half c%2: (512 q, 1024 k).
"""

import numpy as np

BS, NQ, NK = 4, 1024, 1024
QD, KD, VD, HID = 128, 128, 128, 64
NCORES = 8
NQH = NQ // 2  # queries per core

J = 6
W0 = 0.250610
FOURIER_W = [(2 * j + 1) * W0 for j in range(J)]
FOURIER_C = [1.2414016, 0.34017008, 0.14252683, 0.063769671,
             0.029062967, 0.012471759]

TWO_PI = 6.283185307179586
HALF_PI = 1.5707963267948966
PI64 = 64 * 3.141592653589793

_CACHED = {}


def _build():
    import concourse.bacc as bacc
    import concourse.mybir as mybir
    from concourse import tile
    from concourse.alu_op_type import AluOpType
    from concourse.masks import make_identity

    F32 = mybir.dt.float32
    U32 = mybir.dt.uint32
    BF16 = mybir.dt.bfloat16
    A = mybir.ActivationFunctionType

    nc = bacc.Bacc(None, target_bir_lowering=False)

    q_sh = nc.declare_dram_parameter("q_sh", [NQH, QD], F32, isOutput=False)
    k_sh = nc.declare_dram_parameter("k_sh", [NK, KD], F32, isOutput=False)
    v_sh = nc.declare_dram_parameter("v_sh", [NK, VD], F32, isOutput=False)
    NCC = 128 + 128 + J + 5
    cpack = nc.declare_dram_parameter("cpack", [128, NCC], F32, isOutput=False)
    out = nc.declare_dram_parameter("out", [NQH, VD], F32, isOutput=True)

    NQC = NQH // 128  # 4 query chunks
    NKC = NK // 128   # 8 key chunks

    with tile.TileContext(nc) as tc:
        with (
            tc.tile_pool(name="consts", bufs=1) as consts,
            tc.tile_pool(name="io", bufs=1) as io,
            tc.tile_pool(name="chunks", bufs=6) as chunks,
            tc.tile_pool(name="vals", bufs=NKC) as vals,
            tc.tile_pool(name="work", bufs=3) as work,
            tc.tile_pool(name="jbank", bufs=3) as jbank,
            tc.tile_pool(name="sm", bufs=NKC) as sm,
            tc.tile_pool(name="ps", bufs=8, space="PSUM") as ps,
        ):
            # ---- inputs first: q/k spread across queues, transpose to (d, n) --
            id32 = consts.tile([128, 128], F32, tag="id32")
            make_identity(nc, id32[:])
            id16 = consts.tile([128, 128], BF16, tag="id16")
            make_identity(nc, id16[:])
            ones16 = consts.tile([128, 1], BF16, tag="ones16")
            nc.gpsimd.memset(ones16[:], 1.0)
            # warm the Sin activation table set during the DMA phase
            warm = consts.tile([1, 1], F32, tag="warm")
            nc.scalar.activation(warm[:], ones16[:1, :1], A.Sin)
            qT = io.tile([QD, NQH], F32, tag="qT")
            kT = io.tile([KD, NK], F32, tag="kT")
            # transposes share psum tiles (4 blocks each), one wide copy per tile
            p_q = ps.tile([128, 512], F32, tag="t512", name="p_q")
            for i in range(NQC):
                qc = chunks.tile([128, QD], F32, tag="qc")
                nc.sync.dma_start(qc[:], q_sh[i * 128:(i + 1) * 128, :])
                nc.tensor.transpose(p_q[:, i * 128:(i + 1) * 128], qc[:], id32[:])
            nc.vector.tensor_copy(qT[:], p_q[:])
            for h in range(2):
                p_k = ps.tile([128, 512], F32, tag="t512", name=f"p_k_{h}")
                for c in range(4):
                    i = h * 4 + c
                    kc_t = chunks.tile([128, KD], F32, tag="kc")
                    # split the k loads across both HWDGE queues
                    eng = nc.scalar if i % 2 == 0 else nc.sync
                    eng.dma_start(kc_t[:], k_sh[i * 128:(i + 1) * 128, :])
                    nc.tensor.transpose(p_k[:, c * 128:(c + 1) * 128], kc_t[:],
                                        id32[:])
                nc.vector.tensor_copy(kT[:, h * 512:(h + 1) * 512], p_k[:])

            # ---- constants: one packed DMA on the gpsimd queue ----
            cpack_sb = consts.tile([128, NCC], F32, tag="cpack")
            nc.gpsimd.dma_start(cpack_sb[:], cpack[:])
            wq2_sb = cpack_sb[:, 0:128]
            wk2_sb = cpack_sb[:, 128:256]
            cw_sb = cpack_sb[:, 256:256 + J]
            sphq_sb = cpack_sb[:, 256 + J:257 + J]
            sphk_sb = cpack_sb[:, 257 + J:258 + J]
            biasq_sb = cpack_sb[:, 258 + J:259 + J]
            biask_sb = cpack_sb[:, 259 + J:260 + J]
            bias65_sb = cpack_sb[:, 260 + J:261 + J]
            # values: needed only at the tail; own queue. The bf16 casts are
            # emitted late (before the out matmuls) so they don't occupy the
            # DVE queue ahead of the critical kp2/qp2 staging copies.
            v32 = []
            for i in range(NKC):
                vc = vals.tile([128, VD], F32, tag="v32", name=f"v32_{i}")
                nc.gpsimd.dma_start(vc[:], v_sh[i * 128:(i + 1) * 128, :])
                v32.append(vc)

            # ---- projections: packed (2x64 h, n) = [W | W]^T @ xT ----
            qp2 = io.tile([128, NQH], F32, tag="qp2")
            kp2 = io.tile([128, NK], F32, tag="kp2")
            p = ps.tile([128, 512], F32, tag="t512")
            nc.tensor.matmul(p[:], wq2_sb, qT[:], start=True, stop=True)
            nc.vector.tensor_copy(qp2[:], p[:])
            for c in range(2):
                p = ps.tile([128, 512], F32, tag="t512")
                nc.tensor.matmul(p[:], wk2_sb, kT[:, c * 512:(c + 1) * 512],
                                 start=True, stop=True)
                nc.vector.tensor_copy(kp2[:, c * 512:(c + 1) * 512], p[:])

            # ---- per-j banks + transposed score accumulation over j ----
            # Q rows [sin | cos] scaled by c_j*w_v (bf16); K rows [cos | sin].
            psT = [ps.tile([128, 512], F32, tag="t512", name=f"psT_{kt}")
                   for kt in range(NKC)]

            for j in range(J):
                ks = jbank.tile([128, NK], BF16, tag="ks", name=f"ks_{j}")
                sq_f = work.tile([128, NQH], F32, tag="sqf", name=f"sqf_{j}")
                sq = jbank.tile([128, NQH], BF16, tag="sq", name=f"sq_{j}")
                if j == 0:  # |w0 x + pi/2| < 2.9: direct activation
                    nc.scalar.activation(ks[:], kp2[:], A.Sin,
                                         bias=biask_sb, scale=W0)
                    nc.scalar.activation(sq_f[:], qp2[:], A.Sin,
                                         bias=biasq_sb, scale=W0)
                else:
                    s1 = float(FOURIER_W[j] / TWO_PI)
                    zk = work.tile([128, NK], F32, tag="zk", name=f"zk_{j}")
                    vk = work.tile([128, NK], F32, tag="vk", name=f"vk_{j}")
                    nc.vector.tensor_scalar(zk[:], kp2[:], s1, sphk_sb,
                                            AluOpType.mult, AluOpType.add)
                    nc.vector.tensor_scalar(vk[:].bitcast(U32), zk[:].bitcast(U32),
                                            0x0003FFFF, 0x3F800000,
                                            AluOpType.bitwise_and,
                                            AluOpType.bitwise_or)
                    nc.scalar.activation(ks[:], vk[:], A.Sin, scale=-PI64,
                                         bias=bias65_sb)
                    zq = work.tile([128, NQH], F32, tag="zq", name=f"zq_{j}")
                    vq = work.tile([128, NQH], F32, tag="vq", name=f"vq_{j}")
                    nc.vector.tensor_scalar(zq[:], qp2[:], s1, sphq_sb,
                                            AluOpType.mult, AluOpType.add)
                    nc.vector.tensor_scalar(vq[:].bitcast(U32), zq[:].bitcast(U32),
                                            0x0003FFFF, 0x3F800000,
                                            AluOpType.bitwise_and,
                                            AluOpType.bitwise_or)
                    nc.scalar.activation(sq_f[:], vq[:], A.Sin, scale=-PI64,
                                         bias=bias65_sb)
                # c_j*w_v scaling + bf16 cast on ScalarE (Copy with AP scale)
                nc.scalar.mul(sq[:], sq_f[:], cw_sb[:, j:j + 1])
                for kt in range(NKC):
                    nc.tensor.matmul(psT[kt][:],
                                     ks[:, kt * 128:(kt + 1) * 128], sq[:],
                                     start=(j == 0), stop=(j == J - 1))

            # ---- exp (k-major) + denominators via ones-matmul ----
            expT = []
            for kt in range(NKC):
                et = sm.tile([128, 512], BF16, tag="expT", name=f"expT_{kt}")
                nc.scalar.activation(et[:], psT[kt][:], A.Exp)
                expT.append(et)
            psum_sums = ps.tile([1, 512], F32, tag="t512", name="psum_sums")
            for kt in range(NKC):
                nc.tensor.matmul(psum_sums[:], ones16[:], expT[kt][:],
                                 start=(kt == 0), stop=(kt == NKC - 1))
            sums_sb = sm.tile([1, 512], F32, tag="sums_sb")
            nc.scalar.copy(sums_sb[:], psum_sums[:])

            v16 = []
            for i in range(NKC):
                vb = vals.tile([128, VD], BF16, tag="v16", name=f"v16_{i}")
                nc.vector.tensor_copy(vb[:], v32[i][:])
                v16.append(vb)
            # ---- outT (v, q) = sum_kt values[kt] (as lhsT) @ expT[kt] ----
            ps_outT = ps.tile([128, 512], F32, tag="t512", name="ps_outT")
            for kt in range(NKC):
                nc.tensor.matmul(ps_outT[:], v16[kt][:], expT[kt][:],
                                 start=(kt == 0), stop=(kt == NKC - 1))
            outT_sb = sm.tile([128, 512], BF16, tag="outT_sb")
            nc.vector.tensor_copy(outT_sb[:], ps_outT[:])

            # ---- transpose back to (q, v) in bf16, normalize, store ----
            for qt in range(NQC):
                pcol = ps.tile([128, 512], F32, tag="t512", name=f"pcol_{qt}")
                # (1,128) row -> (128,1) column via 1-deep matmul against [[1.0]]
                nc.tensor.matmul(pcol[:128, :1],
                                 sums_sb[:1, qt * 128:(qt + 1) * 128],
                                 id32[:1, :1], start=True, stop=True)
                rcol = sm.tile([128, 1], F32, tag="rcol", name=f"rcol_{qt}")
                nc.vector.reciprocal(rcol[:], pcol[:128, :1])
                po = ps.tile([128, 512], BF16, tag="t512", name=f"po_{qt}")
                nc.tensor.transpose(po[:, :128],
                                    outT_sb[:, qt * 128:(qt + 1) * 128], id16[:])
                o_sb = sm.tile([128, VD], F32, tag="osb", name=f"osb_{qt}")
                nc.vector.tensor_scalar_mul(o_sb[:], po[:, :128], rcol[:])
                eng = nc.sync if qt % 2 == 0 else nc.scalar
                eng.dma_start(out[qt * 128:(qt + 1) * 128, :], o_sb[:])

    nc.finalize()
    return nc


def _get_nc():
    if "nc" not in _CACHED:
        _CACHED["nc"] = _build()
    return _CACHED["nc"]


def _make_consts(W_q, W_k, w_v):
    wq2 = np.concatenate([W_q, W_q], axis=1).astype(np.float32)
    wk2 = np.concatenate([W_k, W_k], axis=1).astype(np.float32)
    cw = np.zeros((128, J), np.float32)
    for j in range(J):
        cwj = (FOURIER_C[j] * w_v).astype(np.float32)
        cw[:64, j] = cwj
        cw[64:, j] = cwj
    # wrap-phase consts (turns, +36 so z lands in [32, 64)):
    # Q packing [sin | cos], K packing [cos | sin]
    sphq = np.full((128, 1), 36.0, np.float32)
    sphq[64:] = 36.25
    sphk = np.full((128, 1), 36.25, np.float32)
    sphk[64:] = 36.0
    # direct-path (j=0) activation bias in radians
    biasq = np.zeros((128, 1), np.float32)
    biasq[64:] = HALF_PI
    biask = np.full((128, 1), HALF_PI, np.float32)
    biask[64:] = 0.0
    bias65 = np.full((128, 1), 65 * np.pi, np.float32)
    return np.concatenate(
        [wq2, wk2, cw, sphq, sphk, biasq, biask, bias65], axis=1)


def kernel(queries, keys, values, W_q, W_k, w_v, _trace=False, _trace_kwargs=None):
    from concourse.bass_utils import run_bass_kernel_spmd

    nc = _get_nc()
    cpack = _make_consts(np.asarray(W_q), np.asarray(W_k), np.asarray(w_v))
    queries = np.ascontiguousarray(queries, np.float32)
    keys = np.ascontiguousarray(keys, np.float32)
    values = np.ascontiguousarray(values, np.float32)

    in_maps = []
    for c in range(NCORES):
        b, qh = c // 2, c % 2
        in_maps.append({
            "q_sh": np.ascontiguousarray(queries[b, qh * NQH:(qh + 1) * NQH, :]),
            "k_sh": keys[b],
            "v_sh": values[b],
            "cpack": cpack,
        })

    kwargs = {}
    if _trace:
        kwargs["trace"] = True
        kwargs.update(_trace_kwargs or {})
    res = run_bass_kernel_spmd(nc, in_maps, core_ids=list(range(NCORES)), **kwargs)

    out = np.empty((BS, NQ, VD), np.float32)
    for c in range(NCORES):
        b, qh = c // 2, c % 2
        out[b, qh * NQH:(qh + 1) * NQH, :] = res.results[c]["out"]
    if _trace:
        return out, res
    return out
